# revision 21
# baseline (speedup 1.0000x reference)
"""Bahdanau attention Trainium2 Bass kernel (data-parallel over batch, 8 cores).

reference:
  q_proj = query @ W1_w.T + W1_b                    [B, D]
  k_proj = einsum('bse,de->bsd', keys, W2_w) + W2_b [B, S, D]
  h      = tanh(q_proj[:, None, :] + k_proj)
  scores = einsum('bsd,d->bs', h, V_w[0]) + V_b[0]
  attn   = softmax(scores, axis=1)
  context= einsum('bs,bse->be', attn, keys)
  returns (context, attn)

Shapes: B=64, S=2048, D=E=1024. 8 NeuronCores, 8 batches/core.

Host prep (cheap, <0.5% of FLOPs): q_proj + combined bias, weight transposes.
Device per core: keys transposed on PE (fp32r, 1.5 cyc/row), the 34-GFLOP
keys@W2.T matmul in fp32r at full PE rate, tanh(+bias) fused into the PSUM
evacuation on ScalarE, scores V-dot as M=1 matmuls, softmax without a max
pass (|scores| <= sum|V| < 15, constant bias -16 keeps exp in fp32 range;
softmax is shift-invariant so the result is identical), exp fused into the
scores evacuation with accum_out providing the partial sums, and context as
M=1 matmuls against the batch's key tiles kept in SBUF. The 1/sum
normalization folds into the ACT evacuation of context/attn as a scale.
V_b is mathematically irrelevant (softmax shift invariance).
"""

import sys

if "/opt/trn_rl_repo" not in sys.path:
    sys.path.insert(0, "/opt/trn_rl_repo")

import numpy as np

import concourse.bacc as bacc
import concourse.tile as tile
from concourse import mybir
from concourse.bass_utils import run_bass_kernel_spmd

FP32 = mybir.dt.float32
FP32R = mybir.dt.float32r
AF = mybir.ActivationFunctionType
ALU = mybir.AluOpType
AXL = mybir.AxisListType

NCORES = 8
B, S, D, E = 64, 2048, 1024, 1024
BC = B // NCORES  # batches per core
P = 128
EC = E // P  # 8 e-chunks
DC = D // P  # 8 d-chunks
SW = 512  # s-tile width (one psum bank)
EXP_BIAS = -16.0  # |scores| <= sum|V_w| (~14.8 worst case) => exp() in range

_prog_cache = {}


def _build(bc=BC, s=S):
    """Build + compile the per-core SPMD program."""
    ns = s // SW  # s512 tiles per batch
    nsub = SW // P  # 128-row subtiles per s512 tile
    nk = s // P  # 128-row key chunks per batch (for context)

    nc = bacc.Bacc("TRN2", target_bir_lowering=False)

    keys_d = nc.dram_tensor("keys", [bc, s, E], FP32R, kind="ExternalInput")
    w2t_d = nc.dram_tensor("w2t", [E, D], FP32R, kind="ExternalInput")  # W2_w.T
    qbt_d = nc.dram_tensor("qbt", [D, bc], FP32, kind="ExternalInput")  # (q_proj+b).T
    vt_d = nc.dram_tensor("vt", [D], FP32R, kind="ExternalInput")  # V_w[0]
    id_d = nc.dram_tensor("identity", [P, P], FP32, kind="ExternalInput")
    ctx_d = nc.dram_tensor("ctx", [bc, E], FP32, kind="ExternalOutput")
    attn_d = nc.dram_tensor("attn", [bc, s], FP32, kind="ExternalOutput")

    with tile.TileContext(nc) as tc:
        with (
            tc.tile_pool(name="const", bufs=1) as constp,
            tc.tile_pool(name="kT", bufs=1) as ktp,
            tc.tile_pool(name="knat", bufs=nk + 4) as knatp,
            tc.tile_pool(name="h", bufs=7) as hp,
            tc.tile_pool(name="ex", bufs=1) as exp_,
            tc.tile_pool(name="out", bufs=1) as outp,
            tc.tile_pool(name="small", bufs=4) as smallp,
            tc.tile_pool(name="ps_tr", bufs=4, space="PSUM") as pstr,
            tc.tile_pool(name="ps_k", bufs=2, space="PSUM") as psk,
            tc.tile_pool(name="ps_s", bufs=2, space="PSUM") as pss,
        ):
            # prefetch the very first key tiles before anything else so the
            # PE can start transposing within a few us
            pre_knats = []
            for sub in range(4):
                kn0 = knatp.tile([P, E], FP32R, tag="knat", name="kn0")
                # two half-DMAs so the first e-chunks land (and transposes
                # start) in half the time
                nc.sync.dma_start(
                    kn0[:, 0 : E // 2], keys_d[0, sub * P : (sub + 1) * P, 0 : E // 2]
                )
                nc.sync.dma_start(
                    kn0[:, E // 2 : E], keys_d[0, sub * P : (sub + 1) * P, E // 2 : E]
                )
                pre_knats.append(kn0)

            identf = constp.tile([P, P], FP32)
            nc.sync.dma_start(identf[:], id_d[:])
            ident = constp.tile([P, P], FP32R)
            nc.sync.dma_start(ident[:], id_d[:].bitcast(FP32R))

            w2t_sb = constp.tile([P, EC, D], FP32R)
            for c in range(EC):
                nc.sync.dma_start(w2t_sb[:, c, :], w2t_d[c * P : (c + 1) * P, :])
            qbt_sb = constp.tile([P, DC, bc], FP32)
            nc.sync.dma_start(qbt_sb[:], qbt_d[:].rearrange("(c p) b -> p c b", p=P))
            ebias = constp.tile([1, 1], FP32)
            # EXP_BIAS from the identity corner (=1.0) via ACT scale; keeps
            # GpSimd entirely out of the program
            nc.scalar.activation(
                ebias[:], identf[0:1, 0:1], AF.Copy, scale=EXP_BIAS
            )
            vt_sb = constp.tile([P, DC], FP32R)
            nc.sync.dma_start(vt_sb[:], vt_d[:].rearrange("(c p) -> p c", p=P))

            # keysT for the CURRENT batch, per e-chunk: [e=128, s]
            keysT = [
                ktp.tile([P, s], FP32R, tag=f"kT{c}", name=f"keysT{c}")
                for c in range(EC)
            ]

            knats_by_b = {}
            ex_by_b = {}
            sume_by_b = {}

            def stage_load_transpose(b, i):
                """DMA natural key tiles + PE-transpose + DVE evac into keysT."""
                s0 = i * SW
                if b == 0 and i == 0:
                    knats_by_b.setdefault(0, []).extend(pre_knats)
                else:
                    for sub in range(nsub):
                        kn = knatp.tile([P, E], FP32R, tag="knat", name="kn")
                        # half-DMAs: the first 4 e-chunks' transposes can
                        # start as soon as the first half lands
                        r0 = s0 + sub * P
                        nc.sync.dma_start(
                            kn[:, 0 : E // 2], keys_d[b, r0 : r0 + P, 0 : E // 2]
                        )
                        nc.sync.dma_start(
                            kn[:, E // 2 : E], keys_d[b, r0 : r0 + P, E // 2 : E]
                        )
                        knats_by_b.setdefault(b, []).append(kn)
                knats = knats_by_b[b]
                for c in range(EC):
                    trp = pstr.tile([P, SW], FP32R, tag="trp", name="trp")
                    for sub in range(nsub):
                        nc.tensor.transpose(
                            trp[:, sub * P : (sub + 1) * P],
                            knats[i * nsub + sub][:, c * P : (c + 1) * P],
                            ident[:],
                        )
                    if c % 2 == 0:
                        nc.vector.tensor_copy(keysT[c][:, s0 : s0 + SW], trp[:])
                    else:
                        nc.scalar.activation(
                            keysT[c][:, s0 : s0 + SW], trp[:], AF.Copy
                        )

            def stage_matmul(b, i):
                """kproj matmuls + tanh + scores + exp for one s512 tile."""
                s0 = i * SW
                if i == 0:
                    ex_by_b[b] = exp_.tile([1, s], FP32, tag="ex", name="ex_sb")
                    sume_by_b[b] = smallp.tile([1, ns], FP32, tag="sume", name="sume")
                ex_sb, sume = ex_by_b[b], sume_by_b[b]
                hs = []
                for dd in range(DC):
                    kps = psk.tile([P, SW], FP32, tag="kps", name="kps")
                    for c in range(EC):
                        nc.tensor.matmul(
                            kps[:],
                            w2t_sb[:, c, dd * P : (dd + 1) * P],
                            keysT[c][:, s0 : s0 + SW],
                            start=(c == 0),
                            stop=(c == EC - 1),
                        )
                    h = hp.tile([P, SW], FP32R, tag="h", name="h")
                    nc.scalar.activation(
                        h[:], kps[:], AF.Tanh, bias=qbt_sb[:, dd, b : b + 1]
                    )
                    hs.append(h)
                sps = pss.tile([1, SW], FP32, tag="sps", name="sps")
                for dd in range(DC):
                    nc.tensor.matmul(
                        sps[:],
                        vt_sb[:, dd : dd + 1],
                        hs[dd][:],
                        start=(dd == 0),
                        stop=(dd == DC - 1),
                    )
                nc.scalar.activation(
                    ex_sb[0:1, s0 : s0 + SW],
                    sps[:],
                    AF.Exp,
                    bias=ebias[0:1, 0:1],
                    accum_out=sume[0:1, i : i + 1],
                )

            def batch_tail(b, emit_next=None):
                """1/sum, attn transpose, context matmuls, outputs."""
                ex_sb, sume = ex_by_b.pop(b), sume_by_b.pop(b)
                knats = knats_by_b.pop(b)
                sumall = smallp.tile([1, 1], FP32, tag="sumall")
                nc.vector.tensor_reduce(sumall[:], sume[:], axis=AXL.X, op=ALU.add)
                rs = smallp.tile([1, 1], FP32, tag="rs")
                nc.vector.reciprocal(rs[:], sumall[:])

                atp = pstr.tile([P, nk], FP32, tag="trp", name="atp")
                for k in range(nk):
                    nc.tensor.transpose(
                        atp[:, k : k + 1],
                        ex_sb[0:1, k * P : (k + 1) * P],
                        identf[0:1, 0:1],
                    )
                atn = outp.tile([P, nk], FP32R, tag="atn", name="atn")
                nc.scalar.activation(atn[:], atp[:], AF.Copy)

                chalf = E // SW
                cps = [
                    pss.tile([1, SW], FP32, tag="sps", name=f"cps{hh}")
                    for hh in range(chalf)
                ]
                for k in range(nk):
                    for hh in range(chalf):
                        nc.tensor.matmul(
                            cps[hh][:],
                            atn[:, k : k + 1],
                            knats[k][:, hh * SW : (hh + 1) * SW],
                            start=(k == 0),
                            stop=(k == nk - 1),
                        )
                    if k == nsub - 1 and emit_next is not None:
                        # first nsub knat slots are free now: prefetch the
                        # next batch's first tile + its transposes here
                        emit_next()
                ctx_sb = outp.tile([1, E], FP32, tag="ctx_sb", name="ctx_sb")
                for hh in range(chalf):
                    nc.scalar.activation(
                        ctx_sb[0:1, hh * SW : (hh + 1) * SW],
                        cps[hh][:],
                        AF.Copy,
                        scale=rs[0:1, 0:1],
                    )
                nc.sync.dma_start(ctx_d[b, :], ctx_sb[0:1, :])
                nc.scalar.activation(
                    ex_sb[0:1, :], ex_sb[0:1, :], AF.Copy, scale=rs[0:1, 0:1]
                )
                nc.sync.dma_start(attn_d[b, :], ex_sb[0:1, :])

            # software pipeline: transpose stage runs one s512 tile ahead;
            # across batch boundaries it is nested into the context matmuls
            stages = [(b, i) for b in range(bc) for i in range(ns)]
            stage_load_transpose(*stages[0])
            for idx, (b, i) in enumerate(stages):
                nxt = stages[idx + 1] if idx + 1 < len(stages) else None
                if nxt is not None and nxt[0] == b:
                    stage_load_transpose(*nxt)
                    stage_matmul(b, i)
                else:
                    stage_matmul(b, i)
                    emit = None
                    if nxt is not None:
                        emit = lambda n=nxt: stage_load_transpose(*n)
                    batch_tail(b, emit_next=emit)

    nc.compile()
    return nc


def _get_prog(bc=BC, s=S):
    key = (bc, s)
    if key not in _prog_cache:
        _prog_cache[key] = _build(bc, s)
    return _prog_cache[key]


def kernel(query, keys, W1_w, W1_b, W2_w, W2_b, V_w, V_b, _trace=False):
    query = np.asarray(query, np.float32)
    keys = np.asarray(keys, np.float32)

    nc = _get_prog()

    # host prep: tiny vs the 275-GFLOP device matmul
    qb = (query @ np.asarray(W1_w, np.float32).T + np.asarray(W1_b, np.float32)
          + np.asarray(W2_b, np.float32))                      # [B, D]
    qbt = np.ascontiguousarray(qb.T)                           # [D, B]
    w2t = np.ascontiguousarray(np.asarray(W2_w, np.float32).T) # [E, D]
    vt = np.ascontiguousarray(np.asarray(V_w, np.float32)[0])  # [D]

    in_maps = []
    for c in range(NCORES):
        sl = slice(c * BC, (c + 1) * BC)
        in_maps.append({
            "keys": keys[sl],
            "qbt": np.ascontiguousarray(qbt[:, sl]),
            "w2t": w2t,
            "vt": vt,
            "identity": np.eye(P, dtype=np.float32),
        })

    res = run_bass_kernel_spmd(nc, in_maps, list(range(NCORES)), trace=_trace)
    context = np.concatenate([r["ctx"] for r in res.results], axis=0)
    attn = np.concatenate([r["attn"] for r in res.results], axis=0)
    if _trace:
        kernel.last_exec_time_ns = res.exec_time_ns
        kernel.last_results = res
    return context.astype(np.float32), attn.astype(np.float32)


# revision 23
# speedup vs baseline: 1.0094x; 1.0094x over previous
"""Bahdanau attention Trainium2 Bass kernel (data-parallel over batch, 8 cores).

reference:
  q_proj = query @ W1_w.T + W1_b                    [B, D]
  k_proj = einsum('bse,de->bsd', keys, W2_w) + W2_b [B, S, D]
  h      = tanh(q_proj[:, None, :] + k_proj)
  scores = einsum('bsd,d->bs', h, V_w[0]) + V_b[0]
  attn   = softmax(scores, axis=1)
  context= einsum('bs,bse->be', attn, keys)
  returns (context, attn)

Shapes: B=64, S=2048, D=E=1024. 8 NeuronCores, 8 batches/core.

Host prep (cheap, <0.5% of FLOPs): q_proj + combined bias, weight transposes.
Device per core: keys transposed on PE (fp32r, 1.5 cyc/row), the 34-GFLOP
keys@W2.T matmul in fp32r at full PE rate, tanh(+bias) fused into the PSUM
evacuation on ScalarE, scores V-dot as M=1 matmuls, softmax without a max
pass (|scores| <= sum|V| < 15, constant bias -16 keeps exp in fp32 range;
softmax is shift-invariant so the result is identical), exp fused into the
scores evacuation with accum_out providing the partial sums, and context as
M=1 matmuls against the batch's key tiles kept in SBUF. The 1/sum
normalization folds into the ACT evacuation of context/attn as a scale.
V_b is mathematically irrelevant (softmax shift invariance).
"""

import sys

if "/opt/trn_rl_repo" not in sys.path:
    sys.path.insert(0, "/opt/trn_rl_repo")

import numpy as np

import concourse.bacc as bacc
import concourse.tile as tile
from concourse import mybir
from concourse.bass_utils import run_bass_kernel_spmd

FP32 = mybir.dt.float32
FP32R = mybir.dt.float32r
AF = mybir.ActivationFunctionType
ALU = mybir.AluOpType
AXL = mybir.AxisListType

NCORES = 8
B, S, D, E = 64, 2048, 1024, 1024
BC = B // NCORES  # batches per core
P = 128
EC = E // P  # 8 e-chunks
DC = D // P  # 8 d-chunks
SW = 512  # s-tile width (one psum bank)
EXP_BIAS = -16.0  # |scores| <= sum|V_w| (~14.8 worst case) => exp() in range

_prog_cache = {}


def _build(bc=BC, s=S):
    """Build + compile the per-core SPMD program."""
    ns = s // SW  # s512 tiles per batch
    nsub = SW // P  # 128-row subtiles per s512 tile
    nk = s // P  # 128-row key chunks per batch (for context)

    nc = bacc.Bacc("TRN2", target_bir_lowering=False)

    keys_d = nc.dram_tensor("keys", [bc, s, E], FP32R, kind="ExternalInput")
    w2t_d = nc.dram_tensor("w2t", [E, D], FP32R, kind="ExternalInput")  # W2_w.T
    qbt_d = nc.dram_tensor("qbt", [D, bc], FP32, kind="ExternalInput")  # (q_proj+b).T
    vt_d = nc.dram_tensor("vt", [D], FP32R, kind="ExternalInput")  # V_w[0]
    id_d = nc.dram_tensor("identity", [P, P], FP32, kind="ExternalInput")
    ctx_d = nc.dram_tensor("ctx", [bc, E], FP32, kind="ExternalOutput")
    attn_d = nc.dram_tensor("attn", [bc, s], FP32, kind="ExternalOutput")

    with tile.TileContext(nc) as tc:
        with (
            tc.tile_pool(name="const", bufs=1) as constp,
            tc.tile_pool(name="kT", bufs=1) as ktp,
            tc.tile_pool(name="knat", bufs=nk + 2) as knatp,
            tc.tile_pool(name="h", bufs=10) as hp,
            tc.tile_pool(name="ex", bufs=1) as exp_,
            tc.tile_pool(name="out", bufs=2) as outp,
            tc.tile_pool(name="small", bufs=4) as smallp,
            tc.tile_pool(name="ps_tr", bufs=3, space="PSUM") as pstr,
            tc.tile_pool(name="ps_k", bufs=3, space="PSUM") as psk,
            tc.tile_pool(name="ps_s", bufs=2, space="PSUM") as pss,
        ):
            # prefetch the very first key tiles before anything else so the
            # PE can start transposing within a few us
            pre_knats = []
            for sub in range(4):
                kn0 = knatp.tile([P, E], FP32R, tag="knat", name="kn0")
                # two half-DMAs so the first e-chunks land (and transposes
                # start) in half the time
                nc.sync.dma_start(
                    kn0[:, 0 : E // 2], keys_d[0, sub * P : (sub + 1) * P, 0 : E // 2]
                )
                nc.sync.dma_start(
                    kn0[:, E // 2 : E], keys_d[0, sub * P : (sub + 1) * P, E // 2 : E]
                )
                pre_knats.append(kn0)

            identf = constp.tile([P, P], FP32)
            nc.sync.dma_start(identf[:], id_d[:])
            ident = constp.tile([P, P], FP32R)
            nc.sync.dma_start(ident[:], id_d[:].bitcast(FP32R))

            w2t_sb = constp.tile([P, EC, D], FP32R)
            for c in range(EC):
                nc.sync.dma_start(w2t_sb[:, c, :], w2t_d[c * P : (c + 1) * P, :])
            qbt_sb = constp.tile([P, DC, bc], FP32)
            nc.sync.dma_start(qbt_sb[:], qbt_d[:].rearrange("(c p) b -> p c b", p=P))
            ebias = constp.tile([1, 1], FP32)
            # EXP_BIAS from the identity corner (=1.0) via ACT scale; keeps
            # GpSimd entirely out of the program
            nc.scalar.activation(
                ebias[:], identf[0:1, 0:1], AF.Copy, scale=EXP_BIAS
            )
            vt_sb = constp.tile([P, DC], FP32R)
            nc.sync.dma_start(vt_sb[:], vt_d[:].rearrange("(c p) -> p c", p=P))

            # keysT for the CURRENT batch, per e-chunk: [e=128, s]
            keysT = [
                ktp.tile([P, s], FP32R, tag=f"kT{c}", name=f"keysT{c}")
                for c in range(EC)
            ]

            knats_by_b = {}
            ex_by_b = {}
            sume_by_b = {}

            def stage_load_transpose(b, i):
                """DMA natural key tiles + PE-transpose + DVE evac into keysT."""
                s0 = i * SW
                if b == 0 and i == 0:
                    knats_by_b.setdefault(0, []).extend(pre_knats)
                else:
                    for sub in range(nsub):
                        kn = knatp.tile([P, E], FP32R, tag="knat", name="kn")
                        # half-DMAs: the first 4 e-chunks' transposes can
                        # start as soon as the first half lands
                        r0 = s0 + sub * P
                        nc.sync.dma_start(
                            kn[:, 0 : E // 2], keys_d[b, r0 : r0 + P, 0 : E // 2]
                        )
                        nc.sync.dma_start(
                            kn[:, E // 2 : E], keys_d[b, r0 : r0 + P, E // 2 : E]
                        )
                        knats_by_b.setdefault(b, []).append(kn)
                knats = knats_by_b[b]
                for c in range(EC):
                    trp = pstr.tile([P, SW], FP32R, tag="trp", name="trp")
                    for sub in range(nsub):
                        nc.tensor.transpose(
                            trp[:, sub * P : (sub + 1) * P],
                            knats[i * nsub + sub][:, c * P : (c + 1) * P],
                            ident[:],
                        )
                    if c % 2 == 0:
                        nc.vector.tensor_copy(keysT[c][:, s0 : s0 + SW], trp[:])
                    else:
                        nc.scalar.activation(
                            keysT[c][:, s0 : s0 + SW], trp[:], AF.Copy
                        )

            def stage_matmul(b, i):
                """kproj matmuls + tanh + scores + exp for one s512 tile."""
                s0 = i * SW
                if i == 0:
                    ex_by_b[b] = exp_.tile([1, s], FP32, tag="ex", name="ex_sb")
                    sume_by_b[b] = smallp.tile([1, ns], FP32, tag="sume", name="sume")
                ex_sb, sume = ex_by_b[b], sume_by_b[b]
                hs = []
                for dd in range(DC):
                    kps = psk.tile([P, SW], FP32, tag="kps", name="kps")
                    for c in range(EC):
                        nc.tensor.matmul(
                            kps[:],
                            w2t_sb[:, c, dd * P : (dd + 1) * P],
                            keysT[c][:, s0 : s0 + SW],
                            start=(c == 0),
                            stop=(c == EC - 1),
                        )
                    h = hp.tile([P, SW], FP32R, tag="h", name="h")
                    nc.scalar.activation(
                        h[:], kps[:], AF.Tanh, bias=qbt_sb[:, dd, b : b + 1]
                    )
                    hs.append(h)
                sps = pss.tile([1, SW], FP32, tag="sps", name="sps")
                for dd in range(DC):
                    nc.tensor.matmul(
                        sps[:],
                        vt_sb[:, dd : dd + 1],
                        hs[dd][:],
                        start=(dd == 0),
                        stop=(dd == DC - 1),
                    )
                nc.scalar.activation(
                    ex_sb[0:1, s0 : s0 + SW],
                    sps[:],
                    AF.Exp,
                    bias=ebias[0:1, 0:1],
                    accum_out=sume[0:1, i : i + 1],
                )

            def batch_tail(b, emit_next=None):
                """1/sum, attn transpose, context matmuls, outputs."""
                ex_sb, sume = ex_by_b.pop(b), sume_by_b.pop(b)
                knats = knats_by_b.pop(b)
                sumall = smallp.tile([1, 1], FP32, tag="sumall")
                nc.vector.tensor_reduce(sumall[:], sume[:], axis=AXL.X, op=ALU.add)
                rs = smallp.tile([1, 1], FP32, tag="rs")
                nc.vector.reciprocal(rs[:], sumall[:])

                atp = pstr.tile([P, nk], FP32, tag="trp", name="atp")
                for k in range(nk):
                    nc.tensor.transpose(
                        atp[:, k : k + 1],
                        ex_sb[0:1, k * P : (k + 1) * P],
                        identf[0:1, 0:1],
                    )
                atn = outp.tile([P, nk], FP32R, tag="atn", name="atn")
                nc.scalar.activation(atn[:], atp[:], AF.Copy)

                chalf = E // SW
                cps = [
                    pss.tile([1, SW], FP32, tag="sps", name=f"cps{hh}")
                    for hh in range(chalf)
                ]
                for k in range(nk):
                    for hh in range(chalf):
                        nc.tensor.matmul(
                            cps[hh][:],
                            atn[:, k : k + 1],
                            knats[k][:, hh * SW : (hh + 1) * SW],
                            start=(k == 0),
                            stop=(k == nk - 1),
                        )
                    if k == nsub - 1 and emit_next is not None:
                        # first nsub knat slots are free now: prefetch the
                        # next batch's first tile + its transposes here
                        emit_next()
                ctx_sb = outp.tile([1, E], FP32, tag="ctx_sb", name="ctx_sb")
                for hh in range(chalf):
                    nc.scalar.activation(
                        ctx_sb[0:1, hh * SW : (hh + 1) * SW],
                        cps[hh][:],
                        AF.Copy,
                        scale=rs[0:1, 0:1],
                    )
                nc.sync.dma_start(ctx_d[b, :], ctx_sb[0:1, :])
                nc.scalar.activation(
                    ex_sb[0:1, :], ex_sb[0:1, :], AF.Copy, scale=rs[0:1, 0:1]
                )
                nc.sync.dma_start(attn_d[b, :], ex_sb[0:1, :])

            # software pipeline: transpose stage runs one s512 tile ahead;
            # across batch boundaries it is nested into the context matmuls
            stages = [(b, i) for b in range(bc) for i in range(ns)]
            stage_load_transpose(*stages[0])
            for idx, (b, i) in enumerate(stages):
                nxt = stages[idx + 1] if idx + 1 < len(stages) else None
                if nxt is not None and nxt[0] == b:
                    stage_load_transpose(*nxt)
                    stage_matmul(b, i)
                else:
                    stage_matmul(b, i)
                    emit = None
                    if nxt is not None:
                        emit = lambda n=nxt: stage_load_transpose(*n)
                    batch_tail(b, emit_next=emit)

    nc.compile()
    return nc


def _get_prog(bc=BC, s=S):
    key = (bc, s)
    if key not in _prog_cache:
        _prog_cache[key] = _build(bc, s)
    return _prog_cache[key]


def kernel(query, keys, W1_w, W1_b, W2_w, W2_b, V_w, V_b, _trace=False):
    query = np.asarray(query, np.float32)
    keys = np.asarray(keys, np.float32)

    nc = _get_prog()

    # host prep: tiny vs the 275-GFLOP device matmul
    qb = (query @ np.asarray(W1_w, np.float32).T + np.asarray(W1_b, np.float32)
          + np.asarray(W2_b, np.float32))                      # [B, D]
    qbt = np.ascontiguousarray(qb.T)                           # [D, B]
    w2t = np.ascontiguousarray(np.asarray(W2_w, np.float32).T) # [E, D]
    vt = np.ascontiguousarray(np.asarray(V_w, np.float32)[0])  # [D]

    in_maps = []
    for c in range(NCORES):
        sl = slice(c * BC, (c + 1) * BC)
        in_maps.append({
            "keys": keys[sl],
            "qbt": np.ascontiguousarray(qbt[:, sl]),
            "w2t": w2t,
            "vt": vt,
            "identity": np.eye(P, dtype=np.float32),
        })

    res = run_bass_kernel_spmd(nc, in_maps, list(range(NCORES)), trace=_trace)
    context = np.concatenate([r["ctx"] for r in res.results], axis=0)
    attn = np.concatenate([r["attn"] for r in res.results], axis=0)
    if _trace:
        kernel.last_exec_time_ns = res.exec_time_ns
        kernel.last_results = res
    return context.astype(np.float32), attn.astype(np.float32)
